# revision 9
# baseline (speedup 1.0000x reference)
"""MultiHeadAttention (RoPE, causal) Trainium2 kernel over 8 NeuronCores.

Sharding: batch (2) x head-groups (4 heads each) -> 8 cores.
Each core computes, for its batch b and 4 heads:
  Q^T,K^T = (Wq/Wk chunk)^T @ x^T   (RoPE applied on-chip)
  S^T tiles = K^T_tile contract-d Q^T, exp (no max-sub; scores ~N(0,1)),
  causal mask via precomputed 0/1 tiles,
  O^T = V contract-k P^T; row-sums l via ones-matmul into a per-j psum
  bank shared by all 4 heads (rows at partitions 0/32/64/96).
  Normalization per (j,h): 1/l on DVE (vector.reciprocal), broadcast
  over partitions with a K=1 matmul, one fused multiply writes ot.
  The output projection for q-tiles of block j runs right after block
  j's attention, so PE never idles at a phase boundary and the z DMA
  spreads across the whole attention phase.
Host sums the 4 per-core partials of each batch.

All matmul operands are bf16 (PSUM accumulation fp32): same 1 cyc/row
stream rate as fp32r but FWL halves the per-matmul LDWEIGHTS cost, the
N>=256 full-rate floor disappears (crossing tiles shrink to exact
multiples of 128), DVE gets 2x modes and DMA bytes halve.  Inputs are
pre-cast to bf16 on host.
Phases share one 8-bank PSUM pool and one x-tile pool (A's projection
sweeps and B's V sweep) so DMA prefetch flows across phase boundaries.
"""

import sys

if "/opt/trn_rl_repo" not in sys.path:
    sys.path.insert(0, "/opt/trn_rl_repo")

import numpy as np

EMBED = 2048
S = 2048
NH = 16
HD = 128
B = 2
N_CORES = 8
HPC = 4              # heads per core
CW = HPC * HD        # 512: per-core projection width
SBK = 512            # s block width
NSB = S // SBK       # 4
NEC = EMBED // 128   # 16 e-chunks
NST = S // 128       # 16 s tiles / q tiles / k tiles
ROPE_BASE = 10000.0
SCALE = 1.0 / float(np.sqrt(HD))

_CACHE = {}


def _build_program():
    import concourse.bacc as bacc
    import concourse.mybir as mybir
    import concourse.tile as tile

    f32 = mybir.dt.float32
    bf16 = mybir.dt.bfloat16
    EXP = mybir.ActivationFunctionType.Exp

    nc = bacc.Bacc("TRN2", target_bir_lowering=False, debug=False,
                   num_devices=N_CORES)

    xt_d = nc.dram_tensor("xt", [EMBED, S], bf16, kind="ExternalInput").ap()
    wq_d = nc.dram_tensor("wq", [EMBED, CW], bf16, kind="ExternalInput").ap()
    wk_d = nc.dram_tensor("wk", [EMBED, CW], bf16, kind="ExternalInput").ap()
    wv_d = nc.dram_tensor("wv", [EMBED, CW], bf16, kind="ExternalInput").ap()
    wo_d = nc.dram_tensor("wo", [CW, EMBED], bf16, kind="ExternalInput").ap()
    cos_d = nc.dram_tensor("cost", [HD, S], bf16, kind="ExternalInput").ap()
    sin_d = nc.dram_tensor("sints", [HD, S], bf16, kind="ExternalInput").ap()
    msk_d = nc.dram_tensor("masks", [128, 4 * SBK], bf16, kind="ExternalInput").ap()
    z_d = nc.dram_tensor("z", [S, EMBED], f32, kind="ExternalOutput").ap()

    XG = 8           # x tiles per s-block (2 e-chunks each)
    XW = 2 * SBK     # x tile width

    with tile.TileContext(nc, pool_alloc_mode="queue") as tc, \
         nc.allow_low_precision(reason="bf16 attention pipeline"):
        pp = tc.alloc_tile_pool(name="persist", bufs=1)
        ps = tc.alloc_tile_pool(name="ps", bufs=8, space="PSUM")
        qt = pp.tile([128, HPC * S], bf16, tag="qt")   # Q^T rope, per head
        kt = pp.tile([128, HPC * S], bf16, tag="kt")   # K^T rope, per head
        # x tiles shared by phases A and B
        xa = tc.alloc_tile_pool(name="xa", bufs=XG)
        # wv pool allocated up front (fresh ring space) so its prefetch
        # DMAs can run under phase A's compute
        wvp = tc.alloc_tile_pool(name="wv", bufs=1)
        wv_sb = wvp.tile([128, NEC * CW], bf16, tag="wv")

        def emit_wv_chunk(g):
            nc.gpsimd.dma_start(
                wv_sb[:, g * 2 * CW:(g + 1) * 2 * CW].rearrange(
                    "p (c m) -> p c m", m=CW),
                wv_d[g * 256:(g + 1) * 256, :].rearrange(
                    "(c p) m -> p c m", p=128))

        # ---------------- Phase A: Q/K projections + RoPE ----------------
        # Per s-block: load 8 x-tiles (held in SBUF), sweep Q over all
        # e-chunks, then sweep K reusing the same x-tiles.  Q's RoPE
        # (DVE) hides under the K sweep and vice versa.
        wp = tc.alloc_tile_pool(name="wqk", bufs=1)
        cs = tc.alloc_tile_pool(name="cossin", bufs=2)
        rp = tc.alloc_tile_pool(name="ropetmp", bufs=1)
        wq_sb = wp.tile([128, NEC * CW], bf16, tag="wq")
        wk_sb = wp.tile([128, NEC * CW], bf16, tag="wk")

        # PE warmup: ~10 dummy matmuls on zeroed SBUF keep the tensor
        # engine busy while the first x/wq DMAs land, so the HAM clock
        # gate opens to 2.4GHz before the real sweeps begin
        wu_sb = rp.tile([128, SBK], f32, tag="wu")
        nc.vector.memset(wu_sb[:], 0.0)
        wu_ps = ps.tile([128, SBK], f32, tag="ps", name="wups")
        for _r in range(2):
            # plain fp32 runs at 4 cyc/row: ~1.7us of PE busy per matmul
            nc.tensor.matmul(wu_ps[0:1, :], lhsT=wu_sb[:, 0:1],
                             rhs=wu_sb[:], start=True, stop=True)

        def rope(psrc, dst, cos_sb, sin_sb, sb, h, nm):
            # ACT drains the psum bank to bf16 (frees it for the next
            # sweep); a small sbuf->sbuf DMA builds the half-swapped copy
            # so every DVE op is same-partition all-bf16 (2x mode, no
            # PSUM access penalty).
            qr = rp.tile([128, SBK], bf16, tag="qr", name=f"qr{nm}{sb}_{h}")
            nc.scalar.copy(qr[:], psrc[:])
            qs = rp.tile([128, SBK], bf16, tag="qs", name=f"qs{nm}{sb}_{h}")
            nc.sync.dma_start(qs[0:64, :], qr[64:128, :])
            nc.sync.dma_start(qs[64:128, :], qr[0:64, :])
            t1 = rp.tile([128, SBK], bf16, tag="t1", name=f"t1{nm}{sb}_{h}")
            t2 = rp.tile([128, SBK], bf16, tag="t2", name=f"t2{nm}{sb}_{h}")
            nc.vector.tensor_mul(t1[:], qr[:], cos_sb[:])
            nc.vector.tensor_mul(t2[:], qs[:], sin_sb[:])
            ds = slice(h * S + sb * SBK, h * S + (sb + 1) * SBK)
            nc.vector.tensor_add(dst[:, ds], t1[:], t2[:])

        wv_sched = {2: [0, 1, 2, 3], 3: [4, 5, 6, 7]}
        for sb in range(NSB):
            for g_wv in wv_sched.get(sb, []):
                emit_wv_chunk(g_wv)  # wv prefetch spread over phase A
            ss = slice(sb * SBK, (sb + 1) * SBK)
            xts = []
            for g in range(XG):
                xt_g = xa.tile([128, XW], bf16, tag="x", name=f"x{sb}_{g}")
                src_ap = xt_d[g * 256:(g + 1) * 256, ss]
                nc.sync.dma_start(
                    xt_g[:].rearrange("p (c s) -> p c s", s=SBK),
                    src_ap.rearrange("(c p) s -> p c s", p=128))
                xts.append(xt_g)
                if sb == 0:
                    # interleave weight loads so the first matmul starts early
                    nc.gpsimd.dma_start(
                        wq_sb[:, g * 2 * CW:(g + 1) * 2 * CW].rearrange(
                            "p (c m) -> p c m", m=CW),
                        wq_d[g * 256:(g + 1) * 256, :].rearrange(
                            "(c p) m -> p c m", p=128))
            # -------- Q sweep --------
            qp = [ps.tile([128, SBK], f32, tag="ps", name=f"qp{sb}_{_h}")
                  for _h in range(HPC)]
            for g in range(XG):
                for el in range(2):
                    ec = 2 * g + el
                    st, sp = (ec == 0), (ec == NEC - 1)
                    xv = xts[g][:, el * SBK:(el + 1) * SBK]
                    for h in range(HPC):
                        wsl = slice(ec * CW + h * HD, ec * CW + (h + 1) * HD)
                        nc.tensor.matmul(qp[h][:], lhsT=wq_sb[:, wsl],
                                         rhs=xv, start=st, stop=sp)
            cos_sb = cs.tile([128, SBK], bf16, tag="cos", name=f"cos{sb}")
            sin_sb = cs.tile([128, SBK], bf16, tag="sin", name=f"sin{sb}")
            nc.scalar.dma_start(cos_sb[:], cos_d[:, ss])
            nc.scalar.dma_start(sin_sb[:], sin_d[:, ss])
            for h in range(HPC):
                rope(qp[h], qt, cos_sb, sin_sb, sb, h, "q")
            # -------- K sweep --------
            kp = [ps.tile([128, SBK], f32, tag="ps", name=f"kp{sb}_{_h}")
                  for _h in range(HPC)]
            for g in range(XG):
                if sb == 0:
                    nc.gpsimd.dma_start(
                        wk_sb[:, g * 2 * CW:(g + 1) * 2 * CW].rearrange(
                            "p (c m) -> p c m", m=CW),
                        wk_d[g * 256:(g + 1) * 256, :].rearrange(
                            "(c p) m -> p c m", p=128))
                for el in range(2):
                    ec = 2 * g + el
                    st, sp = (ec == 0), (ec == NEC - 1)
                    xv = xts[g][:, el * SBK:(el + 1) * SBK]
                    for h in range(HPC):
                        wsl = slice(ec * CW + h * HD, ec * CW + (h + 1) * HD)
                        nc.tensor.matmul(kp[h][:], lhsT=wk_sb[:, wsl],
                                         rhs=xv, start=st, stop=sp)
            for h in range(HPC):
                rope(kp[h], kt, cos_sb, sin_sb, sb, h, "k")

        rp.release()
        cs.release()
        wp.release()

        # vt + masks live through B and C
        vmp = tc.alloc_tile_pool(name="vtmsk", bufs=1, side="right")
        vt = vmp.tile([128, NST * CW], bf16, tag="vt")
        msk_sb = vmp.tile([128, 4 * SBK], bf16, tag="msk")
        nc.gpsimd.dma_start(msk_sb[:], msk_d[:])
        ones_col = msk_sb[:, 511:512]   # all-ones [128,1]
        ones_row = msk_sb[0:1, 0:128]   # all-ones [1,128]

        # ---------------- Phase B: V projection ----------------
        pre_c0 = None
        for sb in range(NSB):
            if sb == NSB - 1:
                pre_c0 = (ps.tile([128, SBK], f32, tag="ps", name="av0_0"),
                          ps.tile([128, SBK], f32, tag="ps", name="l0_0"))
            ss = slice(sb * SBK, (sb + 1) * SBK)
            vp = [ps.tile([128, CW], f32, tag="ps", name=f"vp{sb}_{_s}")
                  for _s in range(4)]
            xts = []
            for g in range(XG):
                xt_g = xa.tile([128, XW], bf16, tag="x", name=f"xb{sb}_{g}")
                src_ap = xt_d[g * 256:(g + 1) * 256, ss]
                nc.sync.dma_start(
                    xt_g[:].rearrange("p (c s) -> p c s", s=SBK),
                    src_ap.rearrange("(c p) s -> p c s", p=128))
                xts.append(xt_g)
            for g in range(XG):
                for el in range(2):
                    ec = 2 * g + el
                    st, sp = (ec == 0), (ec == NEC - 1)
                    for sub in range(4):
                        nc.tensor.matmul(
                            vp[sub][:],
                            lhsT=xts[g][:, el * SBK + sub * 128:
                                        el * SBK + (sub + 1) * 128],
                            rhs=wv_sb[:, ec * CW:(ec + 1) * CW],
                            start=st, stop=sp)
            for sub in range(4):
                stile = sb * 4 + sub
                nc.scalar.copy(vt[:, stile * CW:stile * CW + CW // 2],
                               vp[sub][:, 0:CW // 2])
                nc.vector.tensor_copy(
                    vt[:, stile * CW + CW // 2:(stile + 1) * CW],
                    vp[sub][:, CW // 2:])

        wvp.release()
        xa.release()

        # ---------- Phase C+D: attention + output proj, per q-block ----------
        # wo prefetches during the first attention block
        otp = tc.alloc_tile_pool(name="otp", bufs=2)
        onp = tc.alloc_tile_pool(name="onp", bufs=1)
        ones2 = onp.tile([1, 128], bf16, tag="one2")
        nc.vector.tensor_copy(ones2[:], ones_row)
        wop = tc.alloc_tile_pool(name="wo", bufs=1)
        wo_sb = wop.tile([128, HPC * EMBED], bf16, tag="wo")
        for h in range(HPC):
            nc.gpsimd.dma_start(
                wo_sb[:, h * EMBED:(h + 1) * EMBED],
                wo_d[h * 128:(h + 1) * 128, :])

        zp = tc.alloc_tile_pool(name="zsb", bufs=2)
        ptp = tc.alloc_tile_pool(name="pts", bufs=8)
        lvp = tc.alloc_tile_pool(name="linv", bufs=4)
        sc = onp.tile([1, SBK], f32, tag="sc")   # approx-recip scratch
        ot_tiles = {}
        carry = [None]   # previous head's deferred normalization

        def make_norm(linv, ot, os_, h, j):
            # deferred one head: the PE reaches this broadcast matmul only
            # after a few of the next head's tiles, so the off-PE reciprocal
            # chain (DVE approx -> ACT bf16 copy) has already finished
            def norm():
                bcps = ps.tile([128, SBK], f32, tag="ps", name=f"bc{h}_{j}")
                nc.tensor.matmul(bcps[:], lhsT=ones2[:], rhs=linv[:],
                                 start=True, stop=True)
                bc_sb = lvp.tile([128, SBK], bf16, tag="bc",
                                 name=f"bs{h}_{j}")
                nc.vector.tensor_copy(bc_sb[:], bcps[:])
                nc.vector.tensor_mul(ot[:, os_], ot[:, os_], bc_sb[:])
            return norm

        def emit_C(j, pre=None):
            nkt = 4 * j + 4  # causal: k tiles 0..4j+3
            ot = otp.tile([128, HPC * SBK], bf16, tag="ot", name=f"ot{j}")
            ot_tiles[j] = ot
            for h in range(HPC):
                if h == 0 and pre is not None:
                    avp, lfull = pre
                else:
                    avp = ps.tile([128, SBK], f32, tag="ps",
                                  name=f"av{h}_{j}")
                    lfull = ps.tile([128, SBK], f32, tag="ps",
                                    name=f"l{h}_{j}")
                lp = lfull[0:1, :]
                qs = slice(h * S + j * SBK, h * S + (j + 1) * SBK)
                os_ = slice(h * SBK, (h + 1) * SBK)
                sps = {}

                def escore(i):
                    o_idx = i - 4 * j
                    # crossing tiles: only q >= 128*o_idx is unmasked
                    q0 = 128 * o_idx if o_idx > 0 else 0
                    sp_t = ps.tile([128, SBK], f32, tag="ps",
                                   name=f"s{h}_{j}_{i}")
                    ks = slice(h * S + i * 128, h * S + (i + 1) * 128)
                    nc.tensor.matmul(sp_t[:, q0:SBK], lhsT=kt[:, ks],
                                     rhs=qt[:, qs.start + q0:qs.stop],
                                     start=True, stop=True)
                    sps[i] = (sp_t, q0)

                escore(0)  # 2-deep lookahead: score i+1 issues before AV_i
                prev_full = None
                prev_pair = None
                for i in range(nkt):
                    if i + 1 < nkt:
                        escore(i + 1)
                    if i == 2 and carry[0] is not None:
                        carry[0]()   # previous head's normalization
                        carry[0] = None
                    sp_t, q0 = sps.pop(i)
                    o_idx = i - 4 * j
                    pt_sb = ptp.tile([128, SBK], bf16, tag="p",
                                     name=f"p{h}_{j}_{i}")
                    nc.scalar.activation(pt_sb[:, q0:SBK],
                                         sp_t[:, q0:SBK], EXP,
                                         scale=SCALE)
                    if o_idx >= 0:  # mask the partially-valid diagonal tile
                        t0 = 128 * o_idx
                        t1 = 128 * o_idx + 128
                        nc.vector.tensor_mul(
                            pt_sb[:, t0:t1], pt_sb[:, t0:t1],
                            msk_sb[:, o_idx * SBK + t0:o_idx * SBK + t1])
                    st, sp = (i == 0), (i == nkt - 1)
                    nc.tensor.matmul(
                        avp[:, q0:SBK],
                        lhsT=vt[:, i * CW + h * HD:i * CW + (h + 1) * HD],
                        rhs=pt_sb[:, q0:SBK], start=st, stop=sp)
                    # row sums: quad-sum full tiles on DVE (bf16 pair +
                    # pair-of-pairs), quartering the ones-matmul rows;
                    # crossing tiles stay individual
                    if o_idx < 0:
                        if prev_full is None:
                            prev_full = pt_sb
                        else:
                            padd = ptp.tile([128, SBK], bf16, tag="p",
                                            name=f"pa{h}_{j}_{i}")
                            nc.vector.tensor_add(padd[:], prev_full[:],
                                                 pt_sb[:])
                            prev_full = None
                            if prev_pair is None:
                                prev_pair = padd
                            else:
                                qadd = ptp.tile([128, SBK], bf16, tag="p",
                                                name=f"qa{h}_{j}_{i}")
                                nc.vector.tensor_add(qadd[:], prev_pair[:],
                                                     padd[:])
                                prev_pair = None
                                nc.tensor.matmul(lp[:], lhsT=ones_col,
                                                 rhs=qadd[:],
                                                 start=(i == 3), stop=False)
                    else:
                        nc.tensor.matmul(lp[:, q0:SBK], lhsT=ones_col,
                                         rhs=pt_sb[:, q0:SBK],
                                         start=(nkt == 4 and i == 0),
                                         stop=(i == nkt - 1))
                # stash unnormalized O^T (frees the psum bank) and start the
                # off-PE reciprocal chain; defer the PE-side broadcast+mul
                nc.vector.tensor_copy(ot[:, os_], avp[:])
                nc.vector.reciprocal_approx_fast(out=sc[:], in_=lp[:])
                linv = lvp.tile([1, SBK], bf16, tag="li", name=f"li{h}_{j}")
                nc.scalar.copy(linv[:], sc[:])
                carry[0] = make_norm(linv, ot, os_, h, j)

        def emit_D(j):
            ot = ot_tiles[j]
            for q_i in range(4 * j, 4 * j + 4):
                z_sb = zp.tile([128, EMBED], f32, tag="zs", name=f"zs{q_i}")
                for eb in range(4):
                    zps = ps.tile([128, SBK], f32, tag="ps",
                                  name=f"z{q_i}_{eb}")
                    for h in range(HPC):
                        nc.tensor.matmul(
                            zps[:],
                            lhsT=ot[:, h * SBK + (q_i - 4 * j) * 128:
                                    h * SBK + (q_i - 4 * j + 1) * 128],
                            rhs=wo_sb[:, h * EMBED + eb * SBK:
                                      h * EMBED + (eb + 1) * SBK],
                            start=(h == 0), stop=(h == HPC - 1))
                    zcopy = (nc.scalar.copy if (j == NSB - 1 and eb % 2)
                             else nc.vector.tensor_copy)
                    zcopy(z_sb[:, eb * SBK:(eb + 1) * SBK], zps[:])
                    nc.sync.dma_start(
                        z_d[q_i * 128:(q_i + 1) * 128,
                            eb * SBK:(eb + 1) * SBK],
                        z_sb[:, eb * SBK:(eb + 1) * SBK])

        # pipeline: each norm rides one head behind; D_j after C_{j+1}
        for j in range(NSB):
            emit_C(j, pre=pre_c0 if j == 0 else None)
            if 1 <= j <= NSB - 2:
                emit_D(j - 1)
        emit_D(NSB - 2)
        carry[0]()   # last head's normalization
        emit_D(NSB - 1)
        lvp.release()
        ptp.release()
        zp.release()
        wop.release()
        onp.release()
        otp.release()
        vmp.release()
        pp.release()
        ps.release()

    nc.compile()
    return nc


def _host_tables():
    inv_freq = 1.0 / (ROPE_BASE ** (np.arange(0, HD, 2, dtype=np.float64) / HD))
    ang = np.arange(S, dtype=np.float64)[:, None] * inv_freq[None, :]  # [S, 64]
    cos = np.cos(ang)
    sin = np.sin(ang)
    cost = np.ascontiguousarray(
        np.concatenate([cos, cos], axis=1).T.astype(np.float32))  # [128, S]
    sints = np.ascontiguousarray(
        np.concatenate([-sin, sin], axis=1).T.astype(np.float32))
    kk = np.arange(128)[:, None]
    qq = np.arange(SBK)[None, :]
    masks = np.zeros((128, 4 * SBK), dtype=np.float32)
    for o in range(4):
        masks[:, o * SBK:(o + 1) * SBK] = (kk <= qq - o * 128).astype(np.float32)
    return cost, sints, masks


def _in_maps(x, Wq, Wk, Wv, Wo):
    import ml_dtypes
    bf = ml_dtypes.bfloat16
    cost, sints, masks = _host_tables()
    cost = cost.astype(bf)
    sints = sints.astype(bf)
    masks = masks.astype(bf)
    maps = []
    for c in range(N_CORES):
        b = c // 4
        h0 = (c % 4) * CW  # column offset of this core's 4 heads
        maps.append({
            "xt": np.ascontiguousarray(x[b].T).astype(bf),
            "wq": np.ascontiguousarray(Wq[:, h0:h0 + CW]).astype(bf),
            "wk": np.ascontiguousarray(Wk[:, h0:h0 + CW]).astype(bf),
            "wv": np.ascontiguousarray(Wv[:, h0:h0 + CW]).astype(bf),
            "wo": np.ascontiguousarray(Wo[h0:h0 + CW, :]).astype(bf),
            "cost": cost,
            "sints": sints,
            "masks": masks,
        })
    return maps


def kernel(x, Wq, Wk, Wv, Wo):
    from concourse.bass_utils import run_bass_kernel_spmd

    x = np.asarray(x, dtype=np.float32)
    Wq = np.asarray(Wq, dtype=np.float32)
    Wk = np.asarray(Wk, dtype=np.float32)
    Wv = np.asarray(Wv, dtype=np.float32)
    Wo = np.asarray(Wo, dtype=np.float32)

    if "nc" not in _CACHE:
        _CACHE["nc"] = _build_program()
    nc = _CACHE["nc"]

    res = run_bass_kernel_spmd(nc, _in_maps(x, Wq, Wk, Wv, Wo),
                               core_ids=list(range(N_CORES)))
    zs = [res.results[c]["z"] for c in range(N_CORES)]
    out = np.empty((B, S, EMBED), dtype=np.float32)
    out[0] = zs[0] + zs[1] + zs[2] + zs[3]
    out[1] = zs[4] + zs[5] + zs[6] + zs[7]
    return out


# revision 13
# speedup vs baseline: 1.0162x; 1.0162x over previous
"""MultiHeadAttention (RoPE, causal) Trainium2 kernel over 8 NeuronCores.

Sharding: batch (2) x head-groups (4 heads each) -> 8 cores.
Each core computes, for its batch b and 4 heads:
  Q^T,K^T = (Wq/Wk chunk)^T @ x^T   (RoPE applied on-chip)
  S^T tiles = K^T_tile contract-d Q^T, exp (no max-sub; scores ~N(0,1)),
  causal mask via precomputed 0/1 tiles,
  O^T = V contract-k P^T; row-sums l via ones-matmul into a per-j psum
  bank shared by all 4 heads (rows at partitions 0/32/64/96).
  Normalization per (j,h): 1/l on DVE (vector.reciprocal), broadcast
  over partitions with a K=1 matmul, one fused multiply writes ot.
  The output projection for q-tiles of block j runs right after block
  j's attention, so PE never idles at a phase boundary and the z DMA
  spreads across the whole attention phase.
Host sums the 4 per-core partials of each batch.

All matmul operands are bf16 (PSUM accumulation fp32): same 1 cyc/row
stream rate as fp32r but FWL halves the per-matmul LDWEIGHTS cost, the
N>=256 full-rate floor disappears (crossing tiles shrink to exact
multiples of 128), DVE gets 2x modes and DMA bytes halve.  Inputs are
pre-cast to bf16 on host.
Phases share one 8-bank PSUM pool and one x-tile pool (A's projection
sweeps and B's V sweep) so DMA prefetch flows across phase boundaries.
"""

import sys

if "/opt/trn_rl_repo" not in sys.path:
    sys.path.insert(0, "/opt/trn_rl_repo")

import numpy as np

EMBED = 2048
S = 2048
NH = 16
HD = 128
B = 2
N_CORES = 8
HPC = 4              # heads per core
CW = HPC * HD        # 512: per-core projection width
SBK = 512            # s block width
NSB = S // SBK       # 4
NEC = EMBED // 128   # 16 e-chunks
NST = S // 128       # 16 s tiles / q tiles / k tiles
ROPE_BASE = 10000.0
SCALE = 1.0 / float(np.sqrt(HD))

_CACHE = {}


def _build_program():
    import concourse.bacc as bacc
    import concourse.mybir as mybir
    import concourse.tile as tile

    f32 = mybir.dt.float32
    bf16 = mybir.dt.bfloat16
    EXP = mybir.ActivationFunctionType.Exp

    nc = bacc.Bacc("TRN2", target_bir_lowering=False, debug=False,
                   num_devices=N_CORES)

    xt_d = nc.dram_tensor("xt", [EMBED, S], bf16, kind="ExternalInput").ap()
    wq_d = nc.dram_tensor("wq", [EMBED, CW], bf16, kind="ExternalInput").ap()
    wk_d = nc.dram_tensor("wk", [EMBED, CW], bf16, kind="ExternalInput").ap()
    wv_d = nc.dram_tensor("wv", [EMBED, CW], bf16, kind="ExternalInput").ap()
    wo_d = nc.dram_tensor("wo", [CW, EMBED], bf16, kind="ExternalInput").ap()
    cos_d = nc.dram_tensor("cost", [HD, S], bf16, kind="ExternalInput").ap()
    sin_d = nc.dram_tensor("sints", [HD, S], bf16, kind="ExternalInput").ap()
    msk_d = nc.dram_tensor("masks", [128, 4 * SBK], bf16, kind="ExternalInput").ap()
    z_d = nc.dram_tensor("z", [S, EMBED], f32, kind="ExternalOutput").ap()

    XG = 8           # x tiles per s-block (2 e-chunks each)
    XW = 2 * SBK     # x tile width

    with tile.TileContext(nc, pool_alloc_mode="queue") as tc, \
         nc.allow_low_precision(reason="bf16 attention pipeline"):
        pp = tc.alloc_tile_pool(name="persist", bufs=1)
        ps = tc.alloc_tile_pool(name="ps", bufs=8, space="PSUM")
        qt = pp.tile([128, HPC * S], bf16, tag="qt")   # Q^T rope, per head
        kt = pp.tile([128, HPC * S], bf16, tag="kt")   # K^T rope, per head
        # x tiles shared by phases A and B; 2 blocks deep so the next
        # block's loads prefetch while this block's sweeps still read
        xa = tc.alloc_tile_pool(name="xa", bufs=2 * XG)
        # wv pool allocated up front (fresh ring space) so its prefetch
        # DMAs can run under phase A's compute
        wvp = tc.alloc_tile_pool(name="wv", bufs=1)
        wv_sb = wvp.tile([128, NEC * CW], bf16, tag="wv")

        def emit_wv_chunk(g):
            nc.gpsimd.dma_start(
                wv_sb[:, g * 2 * CW:(g + 1) * 2 * CW].rearrange(
                    "p (c m) -> p c m", m=CW),
                wv_d[g * 256:(g + 1) * 256, :].rearrange(
                    "(c p) m -> p c m", p=128))

        # ---------------- Phase A: Q/K projections + RoPE ----------------
        # Per s-block: load 8 x-tiles (held in SBUF), sweep Q over all
        # e-chunks, then sweep K reusing the same x-tiles.  Q's RoPE
        # (DVE) hides under the K sweep and vice versa.
        wp = tc.alloc_tile_pool(name="wqk", bufs=1)
        cs = tc.alloc_tile_pool(name="cossin", bufs=2)
        rp = tc.alloc_tile_pool(name="ropetmp", bufs=1)
        wq_sb = wp.tile([128, NEC * CW], bf16, tag="wq")
        wk_sb = wp.tile([128, NEC * CW], bf16, tag="wk")

        # PE warmup: ~10 dummy matmuls on zeroed SBUF keep the tensor
        # engine busy while the first x/wq DMAs land, so the HAM clock
        # gate opens to 2.4GHz before the real sweeps begin
        wu_sb = rp.tile([128, SBK], f32, tag="wu")
        nc.vector.memset(wu_sb[:], 0.0)
        wu_ps = ps.tile([128, SBK], f32, tag="ps", name="wups")
        for _r in range(2):
            # plain fp32 runs at 4 cyc/row: ~1.7us of PE busy per matmul
            nc.tensor.matmul(wu_ps[0:1, :], lhsT=wu_sb[:, 0:1],
                             rhs=wu_sb[:], start=True, stop=True)

        def rope(psrc, dst, cos_sb, sin_sb, sb, h, nm):
            # ACT drains the psum bank to bf16 (frees it for the next
            # sweep); a small sbuf->sbuf DMA builds the half-swapped copy
            # so every DVE op is same-partition all-bf16 (2x mode, no
            # PSUM access penalty).
            qr = rp.tile([128, SBK], bf16, tag="qr", name=f"qr{nm}{sb}_{h}")
            nc.scalar.copy(qr[:], psrc[:])
            qs = rp.tile([128, SBK], bf16, tag="qs", name=f"qs{nm}{sb}_{h}")
            nc.sync.dma_start(qs[0:64, :], qr[64:128, :])
            nc.sync.dma_start(qs[64:128, :], qr[0:64, :])
            t1 = rp.tile([128, SBK], bf16, tag="t1", name=f"t1{nm}{sb}_{h}")
            t2 = rp.tile([128, SBK], bf16, tag="t2", name=f"t2{nm}{sb}_{h}")
            nc.vector.tensor_mul(t1[:], qr[:], cos_sb[:])
            nc.vector.tensor_mul(t2[:], qs[:], sin_sb[:])
            ds = slice(h * S + sb * SBK, h * S + (sb + 1) * SBK)
            nc.vector.tensor_add(dst[:, ds], t1[:], t2[:])

        wv_sched = {2: [0, 1, 2, 3], 3: [4, 5, 6, 7]}
        for sb in range(NSB):
            for g_wv in wv_sched.get(sb, []):
                emit_wv_chunk(g_wv)  # wv prefetch spread over phase A
            ss = slice(sb * SBK, (sb + 1) * SBK)
            xts = []
            for g in range(XG):
                xt_g = xa.tile([128, XW], bf16, tag="x", name=f"x{sb}_{g}")
                src_ap = xt_d[g * 256:(g + 1) * 256, ss]
                nc.sync.dma_start(
                    xt_g[:].rearrange("p (c s) -> p c s", s=SBK),
                    src_ap.rearrange("(c p) s -> p c s", p=128))
                xts.append(xt_g)
                if sb == 0:
                    # interleave weight loads so the first matmul starts early
                    nc.gpsimd.dma_start(
                        wq_sb[:, g * 2 * CW:(g + 1) * 2 * CW].rearrange(
                            "p (c m) -> p c m", m=CW),
                        wq_d[g * 256:(g + 1) * 256, :].rearrange(
                            "(c p) m -> p c m", p=128))
            # -------- Q sweep --------
            qp = [ps.tile([128, SBK], f32, tag="ps", name=f"qp{sb}_{_h}")
                  for _h in range(HPC)]
            for g in range(XG):
                for el in range(2):
                    ec = 2 * g + el
                    st, sp = (ec == 0), (ec == NEC - 1)
                    xv = xts[g][:, el * SBK:(el + 1) * SBK]
                    for h in range(HPC):
                        wsl = slice(ec * CW + h * HD, ec * CW + (h + 1) * HD)
                        nc.tensor.matmul(qp[h][:], lhsT=wq_sb[:, wsl],
                                         rhs=xv, start=st, stop=sp)
            cos_sb = cs.tile([128, SBK], bf16, tag="cos", name=f"cos{sb}")
            sin_sb = cs.tile([128, SBK], bf16, tag="sin", name=f"sin{sb}")
            nc.scalar.dma_start(cos_sb[:], cos_d[:, ss])
            nc.scalar.dma_start(sin_sb[:], sin_d[:, ss])
            for h in range(HPC):
                rope(qp[h], qt, cos_sb, sin_sb, sb, h, "q")
            # -------- K sweep --------
            kp = [ps.tile([128, SBK], f32, tag="ps", name=f"kp{sb}_{_h}")
                  for _h in range(HPC)]
            for g in range(XG):
                if sb == 0:
                    nc.gpsimd.dma_start(
                        wk_sb[:, g * 2 * CW:(g + 1) * 2 * CW].rearrange(
                            "p (c m) -> p c m", m=CW),
                        wk_d[g * 256:(g + 1) * 256, :].rearrange(
                            "(c p) m -> p c m", p=128))
                for el in range(2):
                    ec = 2 * g + el
                    st, sp = (ec == 0), (ec == NEC - 1)
                    xv = xts[g][:, el * SBK:(el + 1) * SBK]
                    for h in range(HPC):
                        wsl = slice(ec * CW + h * HD, ec * CW + (h + 1) * HD)
                        nc.tensor.matmul(kp[h][:], lhsT=wk_sb[:, wsl],
                                         rhs=xv, start=st, stop=sp)
            for h in range(HPC):
                rope(kp[h], kt, cos_sb, sin_sb, sb, h, "k")

        rp.release()
        cs.release()
        wp.release()

        # vt + masks live through B and C
        vmp = tc.alloc_tile_pool(name="vtmsk", bufs=1, side="right")
        vt = vmp.tile([128, NST * CW], bf16, tag="vt")
        msk_sb = vmp.tile([128, 4 * SBK], bf16, tag="msk")
        nc.gpsimd.dma_start(msk_sb[:], msk_d[:])
        ones_col = msk_sb[:, 511:512]   # all-ones [128,1]
        ones_row = msk_sb[0:1, 0:128]   # all-ones [1,128]

        # ---------------- Phase B: V projection ----------------
        pre_c0 = None
        for sb in range(NSB):
            if sb == NSB - 1:
                pre_c0 = (ps.tile([128, SBK], f32, tag="ps", name="av0_0"),
                          ps.tile([128, SBK], f32, tag="ps", name="l0_0"))
            ss = slice(sb * SBK, (sb + 1) * SBK)
            vp = [ps.tile([128, CW], f32, tag="ps", name=f"vp{sb}_{_s}")
                  for _s in range(4)]
            xts = []
            for g in range(XG):
                xt_g = xa.tile([128, XW], bf16, tag="x", name=f"xb{sb}_{g}")
                src_ap = xt_d[g * 256:(g + 1) * 256, ss]
                nc.sync.dma_start(
                    xt_g[:].rearrange("p (c s) -> p c s", s=SBK),
                    src_ap.rearrange("(c p) s -> p c s", p=128))
                xts.append(xt_g)
            for g in range(XG):
                for el in range(2):
                    ec = 2 * g + el
                    st, sp = (ec == 0), (ec == NEC - 1)
                    for sub in range(4):
                        nc.tensor.matmul(
                            vp[sub][:],
                            lhsT=xts[g][:, el * SBK + sub * 128:
                                        el * SBK + (sub + 1) * 128],
                            rhs=wv_sb[:, ec * CW:(ec + 1) * CW],
                            start=st, stop=sp)
            for sub in range(4):
                stile = sb * 4 + sub
                nc.scalar.copy(vt[:, stile * CW:stile * CW + CW // 2],
                               vp[sub][:, 0:CW // 2])
                nc.vector.tensor_copy(
                    vt[:, stile * CW + CW // 2:(stile + 1) * CW],
                    vp[sub][:, CW // 2:])

        wvp.release()
        xa.release()

        # ---------- Phase C+D: attention + output proj, per q-block ----------
        # wo prefetches during the first attention block
        otp = tc.alloc_tile_pool(name="otp", bufs=2)
        onp = tc.alloc_tile_pool(name="onp", bufs=1)
        ones2 = onp.tile([1, 128], bf16, tag="one2")
        nc.vector.tensor_copy(ones2[:], ones_row)
        wop = tc.alloc_tile_pool(name="wo", bufs=1)
        wo_sb = wop.tile([128, HPC * EMBED], bf16, tag="wo")
        for h in range(HPC):
            nc.gpsimd.dma_start(
                wo_sb[:, h * EMBED:(h + 1) * EMBED],
                wo_d[h * 128:(h + 1) * 128, :])

        zp = tc.alloc_tile_pool(name="zsb", bufs=2)
        ptp = tc.alloc_tile_pool(name="pts", bufs=8)
        lvp = tc.alloc_tile_pool(name="linv", bufs=4)
        sc = onp.tile([1, SBK], f32, tag="sc")   # approx-recip scratch
        ot_tiles = {}
        carry = [None]   # previous head's deferred normalization

        def make_norm(linv, ot, os_, h, j):
            # deferred one head: the PE reaches this broadcast matmul only
            # after a few of the next head's tiles, so the off-PE reciprocal
            # chain (DVE approx -> ACT bf16 copy) has already finished
            def norm():
                bcps = ps.tile([128, SBK], f32, tag="ps", name=f"bc{h}_{j}")
                nc.tensor.matmul(bcps[:], lhsT=ones2[:], rhs=linv[:],
                                 start=True, stop=True)
                bc_sb = lvp.tile([128, SBK], bf16, tag="bc",
                                 name=f"bs{h}_{j}")
                nc.vector.tensor_copy(bc_sb[:], bcps[:])
                nc.vector.tensor_mul(ot[:, os_], ot[:, os_], bc_sb[:])
            return norm

        def emit_C(j, pre=None):
            nkt = 4 * j + 4  # causal: k tiles 0..4j+3
            ot = otp.tile([128, HPC * SBK], bf16, tag="ot", name=f"ot{j}")
            ot_tiles[j] = ot
            for h in range(HPC):
                if h == 0 and pre is not None:
                    avp, lfull = pre
                else:
                    avp = ps.tile([128, SBK], f32, tag="ps",
                                  name=f"av{h}_{j}")
                    lfull = ps.tile([128, SBK], f32, tag="ps",
                                    name=f"l{h}_{j}")
                lp = lfull[0:1, :]
                qs = slice(h * S + j * SBK, h * S + (j + 1) * SBK)
                os_ = slice(h * SBK, (h + 1) * SBK)
                sps = {}

                def escore(i):
                    o_idx = i - 4 * j
                    # crossing tiles: only q >= 128*o_idx is unmasked
                    q0 = 128 * o_idx if o_idx > 0 else 0
                    sp_t = ps.tile([128, SBK], f32, tag="ps",
                                   name=f"s{h}_{j}_{i}")
                    ks = slice(h * S + i * 128, h * S + (i + 1) * 128)
                    nc.tensor.matmul(sp_t[:, q0:SBK], lhsT=kt[:, ks],
                                     rhs=qt[:, qs.start + q0:qs.stop],
                                     start=True, stop=True)
                    sps[i] = (sp_t, q0)

                escore(0)  # 2-deep lookahead: score i+1 issues before AV_i
                prev_full = None
                prev_pair = None
                quad_q = []   # (qadd, start_flag, ready_i) deferred lp MMs

                def flush_quads(i, force=False):
                    while quad_q and (force or i - quad_q[0][2] >= 2):
                        qadd, st_q, _ = quad_q.pop(0)
                        nc.tensor.matmul(lp[:], lhsT=ones_col, rhs=qadd[:],
                                         start=st_q, stop=False)

                for i in range(nkt):
                    if i + 1 < nkt:
                        escore(i + 1)
                    if i == 2 and carry[0] is not None:
                        carry[0]()   # previous head's normalization
                        carry[0] = None
                    # all quad MMs must land before the first crossing lp MM
                    # (psum ordering: the start=True quad clears the bank)
                    flush_quads(i, force=(i >= 4 * j))
                    sp_t, q0 = sps.pop(i)
                    o_idx = i - 4 * j
                    pt_sb = ptp.tile([128, SBK], bf16, tag="p",
                                     name=f"p{h}_{j}_{i}")
                    nc.scalar.activation(pt_sb[:, q0:SBK],
                                         sp_t[:, q0:SBK], EXP,
                                         scale=SCALE)
                    if o_idx >= 0:  # mask the partially-valid diagonal tile
                        t0 = 128 * o_idx
                        t1 = 128 * o_idx + 128
                        nc.vector.tensor_mul(
                            pt_sb[:, t0:t1], pt_sb[:, t0:t1],
                            msk_sb[:, o_idx * SBK + t0:o_idx * SBK + t1])
                    st, sp = (i == 0), (i == nkt - 1)
                    nc.tensor.matmul(
                        avp[:, q0:SBK],
                        lhsT=vt[:, i * CW + h * HD:i * CW + (h + 1) * HD],
                        rhs=pt_sb[:, q0:SBK], start=st, stop=sp)
                    # row sums: quad-sum full tiles on DVE (bf16 pair +
                    # pair-of-pairs), quartering the ones-matmul rows;
                    # crossing tiles stay individual
                    if o_idx < 0:
                        if prev_full is None:
                            prev_full = pt_sb
                        else:
                            padd = ptp.tile([128, SBK], bf16, tag="p",
                                            name=f"pa{h}_{j}_{i}")
                            nc.vector.tensor_add(padd[:], prev_full[:],
                                                 pt_sb[:])
                            prev_full = None
                            if prev_pair is None:
                                prev_pair = padd
                            else:
                                qadd = ptp.tile([128, SBK], bf16, tag="p",
                                                name=f"qa{h}_{j}_{i}")
                                nc.vector.tensor_add(qadd[:], prev_pair[:],
                                                     padd[:])
                                prev_pair = None
                                quad_q.append((qadd, i == 3, i))
                    else:
                        nc.tensor.matmul(lp[:, q0:SBK], lhsT=ones_col,
                                         rhs=pt_sb[:, q0:SBK],
                                         start=(nkt == 4 and i == 0),
                                         stop=(i == nkt - 1))
                # stash unnormalized O^T (frees the psum bank) and start the
                # off-PE reciprocal chain; defer the PE-side broadcast+mul
                nc.vector.tensor_copy(ot[:, os_], avp[:])
                nc.vector.reciprocal_approx_fast(out=sc[:], in_=lp[:])
                linv = lvp.tile([1, SBK], bf16, tag="li", name=f"li{h}_{j}")
                nc.scalar.copy(linv[:], sc[:])
                carry[0] = make_norm(linv, ot, os_, h, j)

        def emit_D(j):
            ot = ot_tiles[j]
            for q_i in range(4 * j, 4 * j + 4):
                z_sb = zp.tile([128, EMBED], f32, tag="zs", name=f"zs{q_i}")
                for eb in range(4):
                    zps = ps.tile([128, SBK], f32, tag="ps",
                                  name=f"z{q_i}_{eb}")
                    for h in range(HPC):
                        nc.tensor.matmul(
                            zps[:],
                            lhsT=ot[:, h * SBK + (q_i - 4 * j) * 128:
                                    h * SBK + (q_i - 4 * j + 1) * 128],
                            rhs=wo_sb[:, h * EMBED + eb * SBK:
                                      h * EMBED + (eb + 1) * SBK],
                            start=(h == 0), stop=(h == HPC - 1))
                    zcopy = (nc.scalar.copy if (j == NSB - 1 and eb % 2)
                             else nc.vector.tensor_copy)
                    zcopy(z_sb[:, eb * SBK:(eb + 1) * SBK], zps[:])
                    nc.sync.dma_start(
                        z_d[q_i * 128:(q_i + 1) * 128,
                            eb * SBK:(eb + 1) * SBK],
                        z_sb[:, eb * SBK:(eb + 1) * SBK])

        # pipeline: each norm rides one head behind; D_j after C_{j+1}
        for j in range(NSB):
            emit_C(j, pre=pre_c0 if j == 0 else None)
            if 1 <= j <= NSB - 2:
                emit_D(j - 1)
        emit_D(NSB - 2)
        carry[0]()   # last head's normalization
        emit_D(NSB - 1)
        lvp.release()
        ptp.release()
        zp.release()
        wop.release()
        onp.release()
        otp.release()
        vmp.release()
        pp.release()
        ps.release()

    nc.compile()
    return nc


def _host_tables():
    inv_freq = 1.0 / (ROPE_BASE ** (np.arange(0, HD, 2, dtype=np.float64) / HD))
    ang = np.arange(S, dtype=np.float64)[:, None] * inv_freq[None, :]  # [S, 64]
    cos = np.cos(ang)
    sin = np.sin(ang)
    cost = np.ascontiguousarray(
        np.concatenate([cos, cos], axis=1).T.astype(np.float32))  # [128, S]
    sints = np.ascontiguousarray(
        np.concatenate([-sin, sin], axis=1).T.astype(np.float32))
    kk = np.arange(128)[:, None]
    qq = np.arange(SBK)[None, :]
    masks = np.zeros((128, 4 * SBK), dtype=np.float32)
    for o in range(4):
        masks[:, o * SBK:(o + 1) * SBK] = (kk <= qq - o * 128).astype(np.float32)
    return cost, sints, masks


def _in_maps(x, Wq, Wk, Wv, Wo):
    import ml_dtypes
    bf = ml_dtypes.bfloat16
    cost, sints, masks = _host_tables()
    cost = cost.astype(bf)
    sints = sints.astype(bf)
    masks = masks.astype(bf)
    maps = []
    for c in range(N_CORES):
        b = c // 4
        h0 = (c % 4) * CW  # column offset of this core's 4 heads
        maps.append({
            "xt": np.ascontiguousarray(x[b].T).astype(bf),
            "wq": np.ascontiguousarray(Wq[:, h0:h0 + CW]).astype(bf),
            "wk": np.ascontiguousarray(Wk[:, h0:h0 + CW]).astype(bf),
            "wv": np.ascontiguousarray(Wv[:, h0:h0 + CW]).astype(bf),
            "wo": np.ascontiguousarray(Wo[h0:h0 + CW, :]).astype(bf),
            "cost": cost,
            "sints": sints,
            "masks": masks,
        })
    return maps


def kernel(x, Wq, Wk, Wv, Wo):
    from concourse.bass_utils import run_bass_kernel_spmd

    x = np.asarray(x, dtype=np.float32)
    Wq = np.asarray(Wq, dtype=np.float32)
    Wk = np.asarray(Wk, dtype=np.float32)
    Wv = np.asarray(Wv, dtype=np.float32)
    Wo = np.asarray(Wo, dtype=np.float32)

    if "nc" not in _CACHE:
        _CACHE["nc"] = _build_program()
    nc = _CACHE["nc"]

    res = run_bass_kernel_spmd(nc, _in_maps(x, Wq, Wk, Wv, Wo),
                               core_ids=list(range(N_CORES)))
    zs = [res.results[c]["z"] for c in range(N_CORES)]
    out = np.empty((B, S, EMBED), dtype=np.float32)
    out[0] = zs[0] + zs[1] + zs[2] + zs[3]
    out[1] = zs[4] + zs[5] + zs[6] + zs[7]
    return out


# revision 25
# speedup vs baseline: 1.1512x; 1.1329x over previous
"""MultiHeadAttention (RoPE, causal) Trainium2 kernel over 8 NeuronCores.

Sharding: batch (2) x head-groups (4 heads each) -> 8 cores.
Each core computes, for its batch b and 4 heads:
  Q^T,K^T = (Wq/Wk chunk)^T @ x^T   (RoPE applied on-chip)
  S^T tiles = K^T_tile contract-d Q^T, exp (no max-sub; scores ~N(0,1)),
  causal mask via precomputed 0/1 tiles,
  O^T = V contract-k P^T; row-sums l via ones-matmul into a per-j psum
  bank shared by all 4 heads (rows at partitions 0/32/64/96).
  Normalization per (j,h): 1/l on DVE (vector.reciprocal), broadcast
  over partitions with a K=1 matmul, one fused multiply writes ot.
  The output projection for q-tiles of block j runs right after block
  j's attention, so PE never idles at a phase boundary and the z DMA
  spreads across the whole attention phase.
Host sums the 4 per-core partials of each batch.

All matmul operands are bf16 (PSUM accumulation fp32): same 1 cyc/row
stream rate as fp32r but FWL halves the per-matmul LDWEIGHTS cost, the
N>=256 full-rate floor disappears (crossing tiles shrink to exact
multiples of 128), DVE gets 2x modes and DMA bytes halve.  Inputs are
pre-cast to bf16 on host.
Phases share one 8-bank PSUM pool and one x-tile pool (A's projection
sweeps and B's V sweep) so DMA prefetch flows across phase boundaries.
"""

import sys

if "/opt/trn_rl_repo" not in sys.path:
    sys.path.insert(0, "/opt/trn_rl_repo")

import numpy as np

EMBED = 2048
S = 2048
NH = 16
HD = 128
B = 2
N_CORES = 8
HPC = 4              # heads per core
CW = HPC * HD        # 512: per-core projection width
SBK = 512            # s block width
NSB = S // SBK       # 4
NEC = EMBED // 128   # 16 e-chunks
NST = S // 128       # 16 s tiles / q tiles / k tiles
ROPE_BASE = 10000.0
SCALE = 1.0 / float(np.sqrt(HD))

_CACHE = {}


def _build_program():
    import concourse.bacc as bacc
    import concourse.mybir as mybir
    import concourse.tile as tile

    f32 = mybir.dt.float32
    bf16 = mybir.dt.bfloat16
    EXP = mybir.ActivationFunctionType.Exp

    nc = bacc.Bacc("TRN2", target_bir_lowering=False, debug=False,
                   num_devices=N_CORES)

    xt_d = nc.dram_tensor("xt", [EMBED, S], bf16, kind="ExternalInput").ap()
    wq_d = nc.dram_tensor("wq", [EMBED, CW], bf16, kind="ExternalInput").ap()
    wk_d = nc.dram_tensor("wk", [EMBED, CW], bf16, kind="ExternalInput").ap()
    wv_d = nc.dram_tensor("wv", [EMBED, CW], bf16, kind="ExternalInput").ap()
    wo_d = nc.dram_tensor("wo", [CW, EMBED], bf16, kind="ExternalInput").ap()
    cos_d = nc.dram_tensor("cost", [HD, S], bf16, kind="ExternalInput").ap()
    sin_d = nc.dram_tensor("sints", [HD, S], bf16, kind="ExternalInput").ap()
    msk_d = nc.dram_tensor("masks", [128, 4 * SBK], bf16, kind="ExternalInput").ap()
    z_d = nc.dram_tensor("z", [S, EMBED], f32, kind="ExternalOutput").ap()

    XG = 8           # x tiles per s-block (2 e-chunks each)
    XW = 2 * SBK     # x tile width

    with tile.TileContext(nc, pool_alloc_mode="queue") as tc, \
         nc.allow_low_precision(reason="bf16 attention pipeline"):
        pp = tc.alloc_tile_pool(name="persist", bufs=1)
        psA = tc.alloc_tile_pool(name="psA", bufs=8, space="PSUM")
        qt = pp.tile([128, HPC * S], bf16, tag="qt")   # Q^T rope, per head
        kt = pp.tile([128, HPC * S], bf16, tag="kt")   # K^T rope, per head
        # x tiles shared by phases A and B; 2 blocks deep so the next
        # block's loads prefetch while this block's sweeps still read
        xa = tc.alloc_tile_pool(name="xa", bufs=2 * XG)
        # wv pool allocated up front (fresh ring space) so its prefetch
        # DMAs can run under phase A's compute
        wvp = tc.alloc_tile_pool(name="wv", bufs=1)
        wv_sb = wvp.tile([128, NEC * CW], bf16, tag="wv")

        def emit_wv_chunk(g):
            nc.gpsimd.dma_start(
                wv_sb[:, g * 2 * CW:(g + 1) * 2 * CW].rearrange(
                    "p (c m) -> p c m", m=CW),
                wv_d[g * 256:(g + 1) * 256, :].rearrange(
                    "(c p) m -> p c m", p=128))

        # ---------------- Phase A: Q/K projections + RoPE ----------------
        # Per s-block: load 8 x-tiles (held in SBUF), sweep Q over all
        # e-chunks, then sweep K reusing the same x-tiles.  Q's RoPE
        # (DVE) hides under the K sweep and vice versa.
        wp = tc.alloc_tile_pool(name="wqk", bufs=1)
        cs = tc.alloc_tile_pool(name="cossin", bufs=2)
        rp = tc.alloc_tile_pool(name="ropetmp", bufs=6)
        wq_sb = wp.tile([128, NEC * CW], bf16, tag="wq")
        wk_sb = wp.tile([128, NEC * CW], bf16, tag="wk")

        # PE warmup: ~10 dummy matmuls on zeroed SBUF keep the tensor
        # engine busy while the first x/wq DMAs land, so the HAM clock
        # gate opens to 2.4GHz before the real sweeps begin
        wu_sb = rp.tile([128, SBK], f32, tag="wu")
        nc.vector.memset(wu_sb[:], 0.0)
        wu_ps = psA.tile([128, SBK], f32, tag="ps", name="wups")
        for _r in range(2):
            # plain fp32 runs at 4 cyc/row: ~1.7us of PE busy per matmul
            nc.tensor.matmul(wu_ps[0:1, :], lhsT=wu_sb[:, 0:1],
                             rhs=wu_sb[:], start=True, stop=True)

        def rope(psrc, dst, cos_sb, sin_sb, sb, h, nm):
            # ACT drains the psum bank to bf16 (frees it for the next
            # sweep); a small sbuf->sbuf DMA builds the half-swapped copy
            # so every DVE op is same-partition all-bf16 (2x mode, no
            # PSUM access penalty).
            qr = rp.tile([128, SBK], bf16, tag="qr", name=f"qr{nm}{sb}_{h}")
            nc.scalar.copy(qr[:], psrc[:])
            qs = rp.tile([128, SBK], bf16, tag="qs", name=f"qs{nm}{sb}_{h}")
            # swap DMAs ride the scalar queue: they depend on the ACT copy
            # just above, so they never head-of-line block x prefetch
            nc.scalar.dma_start(qs[0:64, :], qr[64:128, :])
            nc.scalar.dma_start(qs[64:128, :], qr[0:64, :])
            t1 = rp.tile([128, SBK], bf16, tag="t1", name=f"t1{nm}{sb}_{h}")
            t2 = rp.tile([128, SBK], bf16, tag="t2", name=f"t2{nm}{sb}_{h}")
            nc.vector.tensor_mul(t1[:], qr[:], cos_sb[:])
            nc.vector.tensor_mul(t2[:], qs[:], sin_sb[:])
            ds = slice(h * S + sb * SBK, h * S + (sb + 1) * SBK)
            nc.vector.tensor_add(dst[:, ds], t1[:], t2[:])

        wv_sched = {2: [0, 1, 2, 3], 3: [4, 5, 6, 7]}
        for sb in range(NSB):
            for g_wv in wv_sched.get(sb, []):
                emit_wv_chunk(g_wv)  # wv prefetch spread over phase A
            ss = slice(sb * SBK, (sb + 1) * SBK)
            xts = []
            for g in range(XG):
                xt_g = xa.tile([128, XW], bf16, tag="x", name=f"x{sb}_{g}")
                src_ap = xt_d[g * 256:(g + 1) * 256, ss]
                nc.sync.dma_start(
                    xt_g[:].rearrange("p (c s) -> p c s", s=SBK),
                    src_ap.rearrange("(c p) s -> p c s", p=128))
                xts.append(xt_g)
                if sb == 0:
                    # interleave weight loads so the first matmul starts early
                    nc.gpsimd.dma_start(
                        wq_sb[:, g * 2 * CW:(g + 1) * 2 * CW].rearrange(
                            "p (c m) -> p c m", m=CW),
                        wq_d[g * 256:(g + 1) * 256, :].rearrange(
                            "(c p) m -> p c m", p=128))
            # -------- Q sweep --------
            qp = [psA.tile([128, SBK], f32, tag="ps", name=f"qp{sb}_{_h}")
                  for _h in range(HPC)]
            for g in range(XG):
                for el in range(2):
                    ec = 2 * g + el
                    st, sp = (ec == 0), (ec == NEC - 1)
                    xv = xts[g][:, el * SBK:(el + 1) * SBK]
                    for h in range(HPC):
                        wsl = slice(ec * CW + h * HD, ec * CW + (h + 1) * HD)
                        nc.tensor.matmul(qp[h][:], lhsT=wq_sb[:, wsl],
                                         rhs=xv, start=st, stop=sp)
            cos_sb = cs.tile([128, SBK], bf16, tag="cos", name=f"cos{sb}")
            sin_sb = cs.tile([128, SBK], bf16, tag="sin", name=f"sin{sb}")
            nc.scalar.dma_start(cos_sb[:], cos_d[:, ss])
            nc.scalar.dma_start(sin_sb[:], sin_d[:, ss])
            for h in range(HPC):
                rope(qp[h], qt, cos_sb, sin_sb, sb, h, "q")
            # -------- K sweep --------
            kp = [psA.tile([128, SBK], f32, tag="ps", name=f"kp{sb}_{_h}")
                  for _h in range(HPC)]
            for g in range(XG):
                if sb == 0:
                    # sync queue: lands after x0's tiles, before x1's
                    # prefetch, so block-0's critical loads get the HBM
                    nc.sync.dma_start(
                        wk_sb[:, g * 2 * CW:(g + 1) * 2 * CW].rearrange(
                            "p (c m) -> p c m", m=CW),
                        wk_d[g * 256:(g + 1) * 256, :].rearrange(
                            "(c p) m -> p c m", p=128))
                for el in range(2):
                    ec = 2 * g + el
                    st, sp = (ec == 0), (ec == NEC - 1)
                    xv = xts[g][:, el * SBK:(el + 1) * SBK]
                    for h in range(HPC):
                        wsl = slice(ec * CW + h * HD, ec * CW + (h + 1) * HD)
                        nc.tensor.matmul(kp[h][:], lhsT=wk_sb[:, wsl],
                                         rhs=xv, start=st, stop=sp)
            for h in range(HPC):
                rope(kp[h], kt, cos_sb, sin_sb, sb, h, "k")

        rp.release()
        cs.release()
        wp.release()

        # vt + masks live through B and C
        vmp = tc.alloc_tile_pool(name="vtmsk", bufs=1, side="right")
        vt = vmp.tile([128, NST * CW], bf16, tag="vt")
        msk_sb = vmp.tile([128, 4 * SBK], bf16, tag="msk")
        nc.gpsimd.dma_start(msk_sb[:], msk_d[:])
        ones_col = msk_sb[:, 511:512]   # all-ones [128,1]
        ones_row = msk_sb[0:1, 0:128]   # all-ones [1,128]

        # ---------------- Phase B: V projection ----------------
        for sb in range(NSB):
            ss = slice(sb * SBK, (sb + 1) * SBK)
            vp = [psA.tile([128, CW], f32, tag="ps", name=f"vp{sb}_{_s}")
                  for _s in range(4)]
            xts = []
            for g in range(XG):
                xt_g = xa.tile([128, XW], bf16, tag="x", name=f"xb{sb}_{g}")
                src_ap = xt_d[g * 256:(g + 1) * 256, ss]
                nc.sync.dma_start(
                    xt_g[:].rearrange("p (c s) -> p c s", s=SBK),
                    src_ap.rearrange("(c p) s -> p c s", p=128))
                xts.append(xt_g)
            for g in range(XG):
                for el in range(2):
                    ec = 2 * g + el
                    st, sp = (ec == 0), (ec == NEC - 1)
                    for sub in range(4):
                        nc.tensor.matmul(
                            vp[sub][:],
                            lhsT=xts[g][:, el * SBK + sub * 128:
                                        el * SBK + (sub + 1) * 128],
                            rhs=wv_sb[:, ec * CW:(ec + 1) * CW],
                            start=st, stop=sp)
            for sub in range(4):
                stile = sb * 4 + sub
                nc.scalar.copy(vt[:, stile * CW:stile * CW + CW // 2],
                               vp[sub][:, 0:CW // 2])
                nc.vector.tensor_copy(
                    vt[:, stile * CW + CW // 2:(stile + 1) * CW],
                    vp[sub][:, CW // 2:])

        wvp.release()
        xa.release()
        psA.release()
        psC = tc.alloc_tile_pool(name="psC", bufs=1, space="PSUM")

        # ---------- Phase C+D: attention + output proj, per q-block ----------
        # wo prefetches during the first attention block
        otp = tc.alloc_tile_pool(name="otp", bufs=2)
        onp = tc.alloc_tile_pool(name="onp", bufs=1)
        wop = tc.alloc_tile_pool(name="wo", bufs=1)
        wo_sb = wop.tile([128, HPC * EMBED], bf16, tag="wo")
        for h in range(HPC):
            nc.gpsimd.dma_start(
                wo_sb[:, h * EMBED:(h + 1) * EMBED],
                wo_d[h * 128:(h + 1) * 128, :])

        zp = tc.alloc_tile_pool(name="zsb", bufs=2)
        ptp = tc.alloc_tile_pool(name="pts", bufs=8)
        lvp = tc.alloc_tile_pool(name="linv", bufs=4)
        sc = onp.tile([1, SBK], f32, tag="sc")   # approx-recip scratch
        ot_tiles = {}
        carry = [None]   # previous head's deferred normalization

        def make_norm(linv, ot, os_, h, j):
            # deferred one head so the off-PE reciprocal chain has finished;
            # the broadcast runs on the idle gpsimd engine (no PE, no PSUM)
            def norm():
                bc_sb = lvp.tile([128, SBK], bf16, tag="bc",
                                 name=f"bs{h}_{j}")
                nc.gpsimd.partition_broadcast(bc_sb[:], linv[:])
                nc.vector.tensor_mul(ot[:, os_], ot[:, os_], bc_sb[:])
            return norm

        def emit_C(j, pre=None):
            nkt = 4 * j + 4  # causal: k tiles 0..4j+3
            ot = otp.tile([128, HPC * SBK], bf16, tag="ot", name=f"ot{j}")
            ot_tiles[j] = ot
            for h in range(HPC):
                if h == 0 and pre is not None:
                    avp, lfull = pre
                else:
                    avp = ps.tile([128, SBK], f32, tag="ps",
                                  name=f"av{h}_{j}")
                    lfull = ps.tile([128, SBK], f32, tag="ps",
                                    name=f"l{h}_{j}")
                lp = lfull[0:1, :]
                qs = slice(h * S + j * SBK, h * S + (j + 1) * SBK)
                os_ = slice(h * SBK, (h + 1) * SBK)
                sps = {}

                def escore(i):
                    o_idx = i - 4 * j
                    # crossing tiles: only q >= 128*o_idx is unmasked
                    q0 = 128 * o_idx if o_idx > 0 else 0
                    sp_t = ps.tile([128, SBK], f32, tag="ps",
                                   name=f"s{h}_{j}_{i}")
                    ks = slice(h * S + i * 128, h * S + (i + 1) * 128)
                    nc.tensor.matmul(sp_t[:, q0:SBK], lhsT=kt[:, ks],
                                     rhs=qt[:, qs.start + q0:qs.stop],
                                     start=True, stop=True)
                    sps[i] = (sp_t, q0)

                escore(0)  # 2-deep lookahead: score i+1 issues before AV_i
                prev_full = None
                prev_pair = None
                quad_q = []   # (qadd, start_flag, ready_i) deferred lp MMs

                def flush_quads(i, force=False):
                    while quad_q and (force or i - quad_q[0][2] >= 2):
                        qadd, st_q, _ = quad_q.pop(0)
                        nc.tensor.matmul(lp[:], lhsT=ones_col, rhs=qadd[:],
                                         start=st_q, stop=False)

                for i in range(nkt):
                    if i + 1 < nkt:
                        escore(i + 1)
                    if i == 2 and carry[0] is not None:
                        carry[0]()   # previous head's normalization
                        carry[0] = None
                    # all quad MMs must land before the first crossing lp MM
                    # (psum ordering: the start=True quad clears the bank)
                    flush_quads(i, force=(i >= 4 * j))
                    sp_t, q0 = sps.pop(i)
                    o_idx = i - 4 * j
                    pt_sb = ptp.tile([128, SBK], bf16, tag="p",
                                     name=f"p{h}_{j}_{i}")
                    nc.scalar.activation(pt_sb[:, q0:SBK],
                                         sp_t[:, q0:SBK], EXP,
                                         scale=SCALE)
                    if o_idx >= 0:  # mask the partially-valid diagonal tile
                        t0 = 128 * o_idx
                        t1 = 128 * o_idx + 128
                        nc.vector.tensor_mul(
                            pt_sb[:, t0:t1], pt_sb[:, t0:t1],
                            msk_sb[:, o_idx * SBK + t0:o_idx * SBK + t1])
                    st, sp = (i == 0), (i == nkt - 1)
                    nc.tensor.matmul(
                        avp[:, q0:SBK],
                        lhsT=vt[:, i * CW + h * HD:i * CW + (h + 1) * HD],
                        rhs=pt_sb[:, q0:SBK], start=st, stop=sp)
                    # row sums: quad-sum full tiles on DVE (bf16 pair +
                    # pair-of-pairs), quartering the ones-matmul rows;
                    # crossing tiles stay individual
                    if o_idx < 0:
                        if prev_full is None:
                            prev_full = pt_sb
                        else:
                            padd = ptp.tile([128, SBK], bf16, tag="p",
                                            name=f"pa{h}_{j}_{i}")
                            nc.vector.tensor_add(padd[:], prev_full[:],
                                                 pt_sb[:])
                            prev_full = None
                            if prev_pair is None:
                                prev_pair = padd
                            else:
                                qadd = ptp.tile([128, SBK], bf16, tag="p",
                                                name=f"qa{h}_{j}_{i}")
                                nc.vector.tensor_add(qadd[:], prev_pair[:],
                                                     padd[:])
                                prev_pair = None
                                quad_q.append((qadd, i == 3, i))
                    else:
                        nc.tensor.matmul(lp[:, q0:SBK], lhsT=ones_col,
                                         rhs=pt_sb[:, q0:SBK],
                                         start=(nkt == 4 and i == 0),
                                         stop=(i == nkt - 1))
                # reciprocal first (frees the single lall slot for the
                # next head ASAP), then stash unnormalized O^T
                nc.vector.reciprocal_approx_fast(out=sc[:], in_=lp[:])
                linv = lvp.tile([1, SBK], bf16, tag="li", name=f"li{h}_{j}")
                nc.scalar.copy(linv[:], sc[:])
                nc.vector.tensor_copy(ot[:, os_], avp[:])
                carry[0] = make_norm(linv, ot, os_, h, j)

        def emit_D(j):
            ot = ot_tiles[j]
            for q_i in range(4 * j, 4 * j + 4):
                z_sb = zp.tile([128, EMBED], f32, tag="zs", name=f"zs{q_i}")
                for eb in range(4):
                    zps = psC.tile([128, SBK], f32, tag="acc", bufs=3,
                                   name=f"z{q_i}_{eb}")
                    for h in range(HPC):
                        nc.tensor.matmul(
                            zps[:],
                            lhsT=ot[:, h * SBK + (q_i - 4 * j) * 128:
                                    h * SBK + (q_i - 4 * j + 1) * 128],
                            rhs=wo_sb[:, h * EMBED + eb * SBK:
                                      h * EMBED + (eb + 1) * SBK],
                            start=(h == 0), stop=(h == HPC - 1))
                    zcopy = (nc.scalar.copy if eb % 2
                             else nc.vector.tensor_copy)
                    zcopy(z_sb[:, eb * SBK:(eb + 1) * SBK], zps[:])
                    nc.sync.dma_start(
                        z_d[q_i * 128:(q_i + 1) * 128,
                            eb * SBK:(eb + 1) * SBK],
                        z_sb[:, eb * SBK:(eb + 1) * SBK])

        # pipeline: each norm rides one head behind; D_j after C_{j+1}
        for j in range(NSB):
            emit_C(j)
            if 1 <= j <= NSB - 2:
                emit_D(j - 1)
        emit_D(NSB - 2)
        carry[0]()   # last head's normalization
        emit_D(NSB - 1)
        lvp.release()
        ptp.release()
        zp.release()
        wop.release()
        onp.release()
        otp.release()
        vmp.release()
        pp.release()
        psC.release()

    nc.compile()
    return nc


def _host_tables():
    inv_freq = 1.0 / (ROPE_BASE ** (np.arange(0, HD, 2, dtype=np.float64) / HD))
    ang = np.arange(S, dtype=np.float64)[:, None] * inv_freq[None, :]  # [S, 64]
    cos = np.cos(ang)
    sin = np.sin(ang)
    cost = np.ascontiguousarray(
        np.concatenate([cos, cos], axis=1).T.astype(np.float32))  # [128, S]
    sints = np.ascontiguousarray(
        np.concatenate([-sin, sin], axis=1).T.astype(np.float32))
    kk = np.arange(128)[:, None]
    qq = np.arange(SBK)[None, :]
    masks = np.zeros((128, 4 * SBK), dtype=np.float32)
    for o in range(4):
        masks[:, o * SBK:(o + 1) * SBK] = (kk <= qq - o * 128).astype(np.float32)
    return cost, sints, masks


def _in_maps(x, Wq, Wk, Wv, Wo):
    import ml_dtypes
    bf = ml_dtypes.bfloat16
    cost, sints, masks = _host_tables()
    cost = cost.astype(bf)
    sints = sints.astype(bf)
    masks = masks.astype(bf)
    maps = []
    for c in range(N_CORES):
        b = c // 4
        h0 = (c % 4) * CW  # column offset of this core's 4 heads
        maps.append({
            "xt": np.ascontiguousarray(x[b].T).astype(bf),
            "wq": np.ascontiguousarray(Wq[:, h0:h0 + CW]).astype(bf),
            "wk": np.ascontiguousarray(Wk[:, h0:h0 + CW]).astype(bf),
            "wv": np.ascontiguousarray(Wv[:, h0:h0 + CW]).astype(bf),
            "wo": np.ascontiguousarray(Wo[h0:h0 + CW, :]).astype(bf),
            "cost": cost,
            "sints": sints,
            "masks": masks,
        })
    return maps


def kernel(x, Wq, Wk, Wv, Wo):
    from concourse.bass_utils import run_bass_kernel_spmd

    x = np.asarray(x, dtype=np.float32)
    Wq = np.asarray(Wq, dtype=np.float32)
    Wk = np.asarray(Wk, dtype=np.float32)
    Wv = np.asarray(Wv, dtype=np.float32)
    Wo = np.asarray(Wo, dtype=np.float32)

    if "nc" not in _CACHE:
        _CACHE["nc"] = _build_program()
    nc = _CACHE["nc"]

    res = run_bass_kernel_spmd(nc, _in_maps(x, Wq, Wk, Wv, Wo),
                               core_ids=list(range(N_CORES)))
    zs = [res.results[c]["z"] for c in range(N_CORES)]
    out = np.empty((B, S, EMBED), dtype=np.float32)
    out[0] = zs[0] + zs[1] + zs[2] + zs[3]
    out[1] = zs[4] + zs[5] + zs[6] + zs[7]
    return out
